# revision 6
# baseline (speedup 1.0000x reference)
"""Trainium2 Bass kernel for MultiLayerRangeAttention (700 ranges x 250 keys, 2 layers).

Strategy
--------
Data-parallel over the 700 independent ranges: pad to 704, give each of the 8
NeuronCores 88 ranges (44 pairs). Everything on-device is computed in a
"transposed" layout so that softmax reductions become matmul contractions and
all element-wise ops run at partition-base 0:

Per pair of ranges (A, B) we ship hT2 [16, 250]:
  row 0 = x_A, row 1 = x_B, rows 2-7 = [ele|azi] embeds of A,
  rows 8-13 = embeds of B, row 14 = ones (bias), row 15 = zero.

Per layer:
  qk   = W_bd.T @ hT2           -> PSUM [64, 500]   (q in cols 0:250, k in 250:500;
                                                     A rows 0:32, B rows 32:64; SCALE folded into Wq)
  vcol = hT2_chunk.T @ Wv_bd    -> PSUM [125, 16]   ([vA,0,0,vB,1,0,0,1] per 125-chunk)
  scoresT_r chunk c = kT_r_c.T @ qT_r  -> PSUM [125, 500] (chunks side-by-side in free dim)
  expT = exp(scoresT)           -> SBUF (no max subtraction needed; scores in [-4, 5])
  num/den: accumulating matmuls with [v|0]/[0|v]/[ones|0]/[0|ones] stationary cols
           -> PSUM nd [2, 500] = [num_A;num_B | den_A;den_B]
  x2 = num * 1/den  (DVE reciprocal + multiply), written back to hT2 rows 0:2
       (layer 0) or to the output staging tile (layer 1).

Division for layer-1 outputs happens on-device; host only gathers, reshapes,
and strips the 4 padded ranges.
"""

import numpy as np

import concourse.bass as bass
import concourse.bacc as bacc
import concourse.tile as tile
from concourse import mybir
from concourse.bass_utils import run_bass_kernel_spmd

NUM_RANGES = 700
K = 250
HID = 32
NUM_LAYERS = 2
SCALE = 1.0 / (HID ** 0.5)

N_CORES = 8
RANGES_PER_CORE = 88          # 704 padded ranges / 8
PAIRS_PER_CORE = RANGES_PER_CORE // 2   # 44
PAD_RANGES = N_CORES * RANGES_PER_CORE  # 704

F32 = mybir.dt.float32
HALF = K // 2                 # 125

# wconst layout: per layer l, cols [136*l, 136*l+136):
#   [0:64)    Wq_bd  (SCALE and bias folded)
#   [64:128)  Wk_bd  (bias folded)
#   [128:136) Wv_bd  ([vA, 0, 0, vB, ones, 0, 0, ones])
WCOLS = 272


def build_nc() -> bass.Bass:
    nc = bacc.Bacc()
    h_all = nc.dram_tensor("h_all", [PAIRS_PER_CORE, 16, K], F32, kind="ExternalInput")[:]
    wconst = nc.dram_tensor("wconst", [16, WCOLS], F32, kind="ExternalInput")[:]
    out_x = nc.dram_tensor("out_x", [2, PAIRS_PER_CORE * K], F32, kind="ExternalOutput")[:]

    with tile.TileContext(nc) as tc:
        with (
            tc.tile_pool(name="wpool", bufs=1) as wpool,
            tc.tile_pool(name="spool", bufs=1) as spool,
            tc.tile_pool(name="hpool", bufs=3) as hpool,
            tc.tile_pool(name="qksb", bufs=2) as qksb,
            tc.tile_pool(name="vcsb", bufs=2) as vcsb,
            tc.tile_pool(name="expool", bufs=4) as expool,
            tc.tile_pool(name="rpool", bufs=2) as rpool,
            tc.tile_pool(name="qkps", bufs=2, space="PSUM") as qkps,
            tc.tile_pool(name="vcps", bufs=2, space="PSUM") as vcps,
            tc.tile_pool(name="scps", bufs=2, space="PSUM") as scps,
            tc.tile_pool(name="ndps", bufs=2, space="PSUM") as ndps,
        ):
            wsb = wpool.tile([16, WCOLS], F32)
            nc.sync.dma_start(out=wsb, in_=wconst)
            stage = spool.tile([2, PAIRS_PER_CORE * K], F32)

            for g in range(PAIRS_PER_CORE):
                hT2 = hpool.tile([16, K], F32)
                nc.sync.dma_start(out=hT2, in_=h_all[g])

                for l in range(NUM_LAYERS):
                    wb = 136 * l
                    # --- q/k projections (both ranges, bias folded) ---
                    qk_ps = qkps.tile([64, 2 * K], F32)
                    nc.tensor.matmul(qk_ps[:, 0:K], lhsT=wsb[:, wb:wb + 64],
                                     rhs=hT2, start=True, stop=True)
                    nc.tensor.matmul(qk_ps[:, K:2 * K], lhsT=wsb[:, wb + 64:wb + 128],
                                     rhs=hT2, start=True, stop=True)
                    qk_sb = qksb.tile([64, 2 * K], F32)
                    nc.vector.tensor_copy(qk_sb, qk_ps)

                    # --- v in column form + ones, both ranges, both 125-chunks ---
                    vc_ps = vcps.tile([HALF, 16], F32)
                    nc.tensor.matmul(vc_ps[:, 0:8], lhsT=hT2[:, 0:HALF],
                                     rhs=wsb[:, wb + 128:wb + 136], start=True, stop=True)
                    nc.tensor.matmul(vc_ps[:, 8:16], lhsT=hT2[:, HALF:K],
                                     rhs=wsb[:, wb + 128:wb + 136], start=True, stop=True)
                    vc_sb = vcsb.tile([HALF, 16], F32)
                    nc.vector.tensor_copy(vc_sb, vc_ps)

                    # --- scoresT + exp per range ---
                    exp_t = []
                    for r in range(2):
                        b = 32 * r
                        sc_ps = scps.tile([HALF, 2 * K], F32)
                        nc.tensor.matmul(
                            sc_ps[:, 0:K],
                            lhsT=qk_sb[b:b + 32, K:K + HALF],
                            rhs=qk_sb[b:b + 32, 0:K], start=True, stop=True)
                        nc.tensor.matmul(
                            sc_ps[:, K:2 * K],
                            lhsT=qk_sb[b:b + 32, K + HALF:2 * K],
                            rhs=qk_sb[b:b + 32, 0:K], start=True, stop=True)
                        ex = expool.tile([HALF, 2 * K], F32)
                        nc.scalar.activation(ex, sc_ps, mybir.ActivationFunctionType.Exp)
                        exp_t.append(ex)

                    # --- numerator / denominator ---
                    nd_ps = ndps.tile([2, 2 * K], F32)
                    num = nd_ps[:, 0:K]
                    den = nd_ps[:, K:2 * K]
                    # num rows [A; B]
                    nc.tensor.matmul(num, lhsT=vc_sb[:, 0:2], rhs=exp_t[0][:, 0:K],
                                     start=True, stop=False)
                    nc.tensor.matmul(num, lhsT=vc_sb[:, 8:10], rhs=exp_t[0][:, K:2 * K],
                                     start=False, stop=False)
                    nc.tensor.matmul(num, lhsT=vc_sb[:, 2:4], rhs=exp_t[1][:, 0:K],
                                     start=False, stop=False)
                    nc.tensor.matmul(num, lhsT=vc_sb[:, 10:12], rhs=exp_t[1][:, K:2 * K],
                                     start=False, stop=True)
                    # den rows [A; B]
                    nc.tensor.matmul(den, lhsT=vc_sb[:, 4:6], rhs=exp_t[0][:, 0:K],
                                     start=True, stop=False)
                    nc.tensor.matmul(den, lhsT=vc_sb[:, 12:14], rhs=exp_t[0][:, K:2 * K],
                                     start=False, stop=False)
                    nc.tensor.matmul(den, lhsT=vc_sb[:, 6:8], rhs=exp_t[1][:, 0:K],
                                     start=False, stop=False)
                    nc.tensor.matmul(den, lhsT=vc_sb[:, 14:16], rhs=exp_t[1][:, K:2 * K],
                                     start=False, stop=True)

                    rden = rpool.tile([2, K], F32)
                    nc.vector.reciprocal(rden, den)
                    if l == 0:
                        nc.vector.tensor_mul(hT2[0:2, :], num, rden)
                    else:
                        nc.vector.tensor_mul(stage[:, K * g:K * (g + 1)], num, rden)

            nc.sync.dma_start(out=out_x, in_=stage)
    nc.compile()
    return nc


def build_wconst(Wq, bq, Wk, bk, Wv, bv) -> np.ndarray:
    w = np.zeros((16, WCOLS), np.float32)
    for l in range(NUM_LAYERS):
        base = 136 * l
        wq = (Wq[l] * SCALE).astype(np.float32)   # [7, 32]
        bq_l = (bq[l] * SCALE).astype(np.float32)
        wk = Wk[l].astype(np.float32)
        bk_l = bk[l].astype(np.float32)
        wv = Wv[l].astype(np.float32)             # [7, 1]
        bv_l = bv[l].astype(np.float32)           # [1]

        for (mat, bias, off) in ((wq, bq_l, 0), (wk, bk_l, 64)):
            blk = w[:, base + off:base + off + 64]
            blk[0, 0:32] = mat[0]
            blk[2:8, 0:32] = mat[1:7]
            blk[14, 0:32] = bias
            blk[1, 32:64] = mat[0]
            blk[8:14, 32:64] = mat[1:7]
            blk[14, 32:64] = bias

        vb = w[:, base + 128:base + 136]
        vb[0, 0] = wv[0, 0]
        vb[2:8, 0] = wv[1:7, 0]
        vb[14, 0] = bv_l[0]
        vb[1, 3] = wv[0, 0]
        vb[8:14, 3] = wv[1:7, 0]
        vb[14, 3] = bv_l[0]
        vb[14, 4] = 1.0
        vb[14, 7] = 1.0
    return w


def prep_inputs(inputs) -> list[dict]:
    pv = np.ascontiguousarray(np.asarray(inputs["power_vals"], np.float32)).reshape(-1)
    ele = np.asarray(inputs["ele_indices"]).astype(np.int64)
    azi = np.asarray(inputs["azi_indices"]).astype(np.int64)
    e = np.asarray(inputs["ele_emb"], np.float32)[ele]   # [N, 3]
    a = np.asarray(inputs["azi_emb"], np.float32)[azi]   # [N, 3]
    n = NUM_RANGES * K
    feats = np.empty((PAD_RANGES * K, 7), np.float32)
    feats[n:] = 0.0
    feats[:n, 0] = pv
    feats[:n, 1:4] = e
    feats[:n, 4:7] = a
    fT = feats.reshape(PAD_RANGES, K, 7).transpose(0, 2, 1)  # [704, 7, 250]
    npairs = PAD_RANGES // 2
    hT = np.zeros((npairs, 16, K), np.float32)
    A, B = fT[0::2], fT[1::2]
    hT[:, 0] = A[:, 0]
    hT[:, 1] = B[:, 0]
    hT[:, 2:8] = A[:, 1:7]
    hT[:, 8:14] = B[:, 1:7]
    hT[:, 14] = 1.0
    hT = np.ascontiguousarray(hT)

    wconst = build_wconst(inputs["Wq"], inputs["bq"], inputs["Wk"], inputs["bk"],
                          inputs["Wv"], inputs["bv"])
    return [
        {"h_all": np.ascontiguousarray(hT[PAIRS_PER_CORE * c:PAIRS_PER_CORE * (c + 1)]),
         "wconst": wconst}
        for c in range(N_CORES)
    ]


_NC_CACHE = None
LAST_RESULTS = None


def _ensure_ntff_hook():
    """The agent image's ``antenv`` lacks ``axon_hooks``, so trn_boot's NTFF
    hook registration degrades silently. Recreate the module + hook here so
    ``run_bass_kernel_spmd(trace=True)`` can capture HW exec times."""
    import sys
    import types
    try:
        from antenv.axon_hooks import get_axon_ntff_profile_hook  # noqa: F401
        return  # already present
    except ImportError:
        pass
    try:
        from trn_agent_boot.trn_boot import _ntff_profile_via_ctypes
        hook = _ntff_profile_via_ctypes("/opt/axon/libaxon_pjrt.so")
        mod = types.ModuleType("antenv.axon_hooks")
        mod._hook = hook
        mod.get_axon_ntff_profile_hook = lambda: mod._hook
        mod.set_axon_ntff_profile_hook = lambda h: setattr(mod, "_hook", h)
        import antenv
        antenv.axon_hooks = mod
        sys.modules["antenv.axon_hooks"] = mod
    except Exception as ex:  # profiling is best-effort
        print(f"NTFF hook setup failed ({ex}); running without trace", flush=True)


def kernel(**inputs) -> np.ndarray:
    global _NC_CACHE, LAST_RESULTS
    if _NC_CACHE is None:
        _NC_CACHE = build_nc()
    nc = _NC_CACHE
    in_maps = prep_inputs(inputs)

    import os
    trace = bool(os.environ.get("KERNEL_TRACE"))
    if trace:
        _ensure_ntff_hook()
    res = run_bass_kernel_spmd(nc, in_maps, core_ids=list(range(N_CORES)), trace=trace)
    LAST_RESULTS = res

    outs = np.stack([r["out_x"] for r in res.results])       # [8, 2, 44*250]
    x = outs.reshape(N_CORES, 2, PAIRS_PER_CORE, K)          # [c, r, g, k]
    x = x.transpose(0, 2, 1, 3).reshape(PAD_RANGES * K)      # range = 88c + 2g + r
    return np.ascontiguousarray(x[:NUM_RANGES * K].reshape(NUM_RANGES * K, 1)).astype(np.float32)


# revision 24
# speedup vs baseline: 5.0587x; 5.0587x over previous
"""Trainium2 Bass kernel for MultiLayerRangeAttention (700 ranges x 250 keys, 2 layers).

Strategy (v2)
-------------
Data-parallel over ranges: pad 700 -> 704, 88 ranges per core, processed as 22
quads of 4 ranges. All matmuls in bf16 (fp32 matmul is emulated 2-pass on TRN2).
Transposed layout per quad: hT4 [32, 250] holds 4 ranges (8 rows each:
x, 6 embed rows, ones). Per layer l and quad:

  q/k:   W_bd4 [32,128] blockdiag -> PSUM [128, 500] (q | k in free),
         4 ranges stacked 32-partitions apart; copy+cast to bf16.
  vcol:  hT4-chunk lhsT x Wv_bd4 -> [v_r | ones] column pairs in PSUM scratch.
  scores(T): per range r at tile_position (32r, 0) -> 4-way concurrent MMs
         into a 4-bank PSUM mega tile [128, 2048] (range r in bank r).
  exp:   one ACT instr over a strided AP [125, 4, 500] -> bf16 SBUF.
  num/den: accumulating [v|ones] MMs at tile_position (0, 32r) ->
         scratch rows {32r, 32r+1}; DMA-gathered into per-layer staging
         tiles (num rows, den rows) across all 88 ranges.

Phase-structured division: after all 22 quads of a layer, ONE DVE reciprocal
[88, 250] + ONE multiply produce all x2 values; a DMA scatters them back into
the hT4 tiles (layer 0) or out to DRAM (layer 1). This amortizes the expensive
DVE reciprocal (~6.5 cyc/elem) across all ranges.
"""

import numpy as np
import ml_dtypes

import concourse.bass as bass
import concourse.bacc as bacc
import concourse.tile as tile
from concourse.tile import add_dep_helper
from concourse import mybir
from concourse.bass_utils import run_bass_kernel_spmd

NUM_RANGES = 700
K = 250
HID = 32
NUM_LAYERS = 2
SCALE = 1.0 / (HID ** 0.5)

N_CORES = 8
RPC = 88                      # ranges per core
QUADS = RPC // 4              # 22
PAD_RANGES = N_CORES * RPC    # 704

F32 = mybir.dt.float32
BF16 = mybir.dt.bfloat16
HALF = K // 2                 # 125

# wconst layout (bf16): per layer l, cols [384*l, 384*l + 384):
#   [0:128)   Wq_bd4 (SCALE+bias folded)   [128:256) Wk_bd4 (bias folded)
#   [256:384) Wv_bd4_wide: group r at cols [32r, 32r+32) = [v_r | ones_r | 0...]
WCOLS = 768


def build_nc() -> bass.Bass:
    nc = bacc.Bacc()
    h_all = nc.dram_tensor("h_all", [QUADS, 32, K], BF16, kind="ExternalInput")[:]
    wconst = nc.dram_tensor("wconst", [32, WCOLS], BF16, kind="ExternalInput")[:]
    out_x = nc.dram_tensor("out_x", [RPC, K], F32, kind="ExternalOutput")[:]

    with tile.TileContext(nc) as tc:
        with (
            tc.tile_pool(name="wpool", bufs=1) as wpool,
            tc.tile_pool(name="hpool", bufs=QUADS) as hpool,
            tc.tile_pool(name="qksb", bufs=3) as qksb,
            tc.tile_pool(name="vcsb", bufs=3) as vcsb,
            tc.tile_pool(name="expool", bufs=3) as expool,
            tc.tile_pool(name="ndst", bufs=2) as ndst,
            tc.tile_pool(name="ndsbp", bufs=2 * QUADS) as ndsbp,
            tc.tile_pool(name="divp", bufs=2) as divp,
            tc.tile_pool(name="qkps", bufs=2, space="PSUM") as qkps,
            tc.tile_pool(name="scps", bufs=1, space="PSUM") as scps,
            tc.tile_pool(name="srps", bufs=2, space="PSUM") as srps,
        ):
            wsb = wpool.tile([32, WCOLS], BF16)
            nc.sync.dma_start(out=wsb, in_=wconst)

            hts = []
            for q in range(QUADS):
                ht = hpool.tile([32, K], BF16)
                nc.sync.dma_start(out=ht, in_=h_all[q])
                hts.append(ht)

            for l in range(NUM_LAYERS):
                wb = 384 * l
                # num at cols [0:250), den at cols [250:500), one row per range
                nd_all = ndst.tile([96, 2 * K], F32, tag="nd_all")
                for q in range(QUADS):
                    ht = hts[q]
                    # --- q/k projections ---
                    qk_ps = qkps.tile([128, 2 * K], F32)
                    nc.tensor.matmul(qk_ps[:, 0:K], lhsT=wsb[:, wb:wb + 128],
                                     rhs=ht, start=True, stop=True)
                    nc.tensor.matmul(qk_ps[:, K:2 * K], lhsT=wsb[:, wb + 128:wb + 256],
                                     rhs=ht, start=True, stop=True)
                    qk_sb = qksb.tile([128, 2 * K], BF16)
                    nc.vector.tensor_copy(qk_sb, qk_ps)

                    # --- scratch bank: vcol cols [250:506), nd rows at [0:128) ---
                    scratch = srps.tile([128, 512], F32)
                    nc.tensor.matmul(scratch[0:HALF, K:K + 128], lhsT=ht[:, 0:HALF],
                                     rhs=wsb[:, wb + 256:wb + 384], start=True, stop=True)
                    nc.tensor.matmul(scratch[0:HALF, K + 128:K + 256], lhsT=ht[:, HALF:K],
                                     rhs=wsb[:, wb + 256:wb + 384], start=True, stop=True)
                    vc_sb = vcsb.tile([HALF, 256], BF16)
                    nc.vector.tensor_copy(vc_sb, scratch[0:HALF, K:K + 256])

                    # --- scoresT: 4-way concurrent across row groups ---
                    sc_ps = scps.tile([128, 2048], F32)
                    sc_mms = []
                    for c in range(2):
                        for r in range(4):
                            b = 32 * r
                            mm = nc.tensor.matmul(
                                sc_ps[0:HALF, 512 * r + 250 * c:512 * r + 250 * c + 250],
                                lhsT=qk_sb[b:b + 32, K + HALF * c:K + HALF * c + HALF],
                                rhs=qk_sb[b:b + 32, 0:K],
                                start=True, stop=True, tile_position=(b, 0))
                            sc_mms.append(mm)
                    # --- exp (single ACT op over strided PSUM AP) ---
                    ex = expool.tile([HALF, 4, 2 * K], BF16)
                    sc_view = sc_ps[0:HALF, :].rearrange("p (b x) -> p b x", x=512)[:, :, 0:2 * K]
                    act = nc.scalar.activation(ex, sc_view, mybir.ActivationFunctionType.Exp)
                    for mm in sc_mms:
                        add_dep_helper(act.ins, mm.ins, sync=True, reason="exp after scores")

                    # --- num/den: [v | ones | 0...] accumulating MMs, 4-way col groups ---
                    nd_mms = []
                    for r in range(4):
                        for c in range(2):
                            b = 32 * r
                            mm = nc.tensor.matmul(
                                scratch[b:b + 32, 0:K],
                                lhsT=vc_sb[:, 128 * c + b:128 * c + b + 32],
                                rhs=ex[:, r, 250 * c:250 * c + 250],
                                start=(c == 0), stop=(c == 1), tile_position=(0, b))
                            nd_mms.append(mm)
                    # --- evacuate num/den rows: PSUM -> SBUF copy, then ONE DMA gather ---
                    ndsb = ndsbp.tile([128, K], F32)
                    cp = nc.vector.tensor_copy(ndsb, scratch[:, 0:K])
                    for mm in nd_mms:
                        add_dep_helper(cp.ins, mm.ins, sync=True, reason="ndsb copy after nd")
                    for r in range(4):
                        nc.sync.dma_start(out=nd_all[4 * q + r:4 * q + r + 1, :],
                                          in_=ndsb[32 * r:32 * r + 2, :])

                # --- phase: all divisions for this layer at once ---
                rden = divp.tile([RPC, K], F32, tag="rden")
                nc.vector.reciprocal(rden, nd_all[0:RPC, K:2 * K])
                if l == 0:
                    x2b = divp.tile([RPC, K], BF16, tag="x2b")
                    nc.vector.tensor_mul(x2b, nd_all[0:RPC, 0:K], rden)
                    for q in range(QUADS):
                        nc.sync.dma_start(out=hts[q][0:32:8, :],
                                          in_=x2b[4 * q:4 * q + 4, :])
                else:
                    x2f = divp.tile([RPC, K], F32, tag="x2f")
                    nc.vector.tensor_mul(x2f, nd_all[0:RPC, 0:K], rden)
                    nc.sync.dma_start(out=out_x, in_=x2f)
    nc.compile()
    return nc


def build_wconst(Wq, bq, Wk, bk, Wv, bv) -> np.ndarray:
    w = np.zeros((32, WCOLS), np.float32)
    for l in range(NUM_LAYERS):
        base = 384 * l
        wq = Wq[l] * SCALE
        bq_l = bq[l] * SCALE
        for (mat, bias, off) in ((wq, bq_l, 0), (Wk[l], bk[l], 128)):
            blk = w[:, base + off:base + off + 128]
            for r in range(4):
                cols = slice(32 * r, 32 * r + 32)
                blk[8 * r + 0, cols] = mat[0]
                blk[8 * r + 1:8 * r + 7, cols] = mat[1:7]
                blk[8 * r + 7, cols] = bias
        vb = w[:, base + 256:base + 384]
        for r in range(4):
            vb[8 * r + 0, 32 * r] = Wv[l][0, 0]
            vb[8 * r + 1:8 * r + 7, 32 * r] = Wv[l][1:7, 0]
            vb[8 * r + 7, 32 * r] = bv[l][0]
            vb[8 * r + 7, 32 * r + 1] = 1.0
    return w.astype(ml_dtypes.bfloat16)


def prep_inputs(inputs) -> list[dict]:
    pv = np.ascontiguousarray(np.asarray(inputs["power_vals"], np.float32)).reshape(-1)
    ele = np.asarray(inputs["ele_indices"]).astype(np.int64)
    azi = np.asarray(inputs["azi_indices"]).astype(np.int64)
    e = np.asarray(inputs["ele_emb"], np.float32)[ele]   # [N, 3]
    a = np.asarray(inputs["azi_emb"], np.float32)[azi]   # [N, 3]
    n = NUM_RANGES * K
    feats = np.empty((PAD_RANGES * K, 8), np.float32)
    feats[n:] = 0.0
    feats[:n, 0] = pv
    feats[:n, 1:4] = e
    feats[:n, 4:7] = a
    feats[:, 7] = 1.0
    # [704, 250, 8] -> [704, 8, 250] -> quads [176, 4, 8, 250] -> [176, 32, 250]
    fT = feats.reshape(PAD_RANGES, K, 8).transpose(0, 2, 1)
    hT = np.ascontiguousarray(
        fT.reshape(PAD_RANGES // 4, 4 * 8, K)).astype(ml_dtypes.bfloat16)

    wconst = build_wconst(np.asarray(inputs["Wq"], np.float32),
                          np.asarray(inputs["bq"], np.float32),
                          np.asarray(inputs["Wk"], np.float32),
                          np.asarray(inputs["bk"], np.float32),
                          np.asarray(inputs["Wv"], np.float32),
                          np.asarray(inputs["bv"], np.float32))
    return [
        {"h_all": np.ascontiguousarray(hT[QUADS * c:QUADS * (c + 1)]),
         "wconst": wconst}
        for c in range(N_CORES)
    ]


_NC_CACHE = None
LAST_RESULTS = None


def _ensure_ntff_hook():
    """The agent image's ``antenv`` lacks ``axon_hooks``, so trn_boot's NTFF
    hook registration degrades silently. Recreate the module + hook here so
    ``run_bass_kernel_spmd(trace=True)`` can capture HW exec times."""
    import sys
    import types
    try:
        from antenv.axon_hooks import get_axon_ntff_profile_hook  # noqa: F401
        return
    except ImportError:
        pass
    try:
        from trn_agent_boot.trn_boot import _ntff_profile_via_ctypes
        hook = _ntff_profile_via_ctypes("/opt/axon/libaxon_pjrt.so")
        mod = types.ModuleType("antenv.axon_hooks")
        mod._hook = hook
        mod.get_axon_ntff_profile_hook = lambda: mod._hook
        mod.set_axon_ntff_profile_hook = lambda h: setattr(mod, "_hook", h)
        import antenv
        antenv.axon_hooks = mod
        sys.modules["antenv.axon_hooks"] = mod
    except Exception as ex:
        print(f"NTFF hook setup failed ({ex}); running without trace", flush=True)


def kernel(**inputs) -> np.ndarray:
    global _NC_CACHE, LAST_RESULTS
    if _NC_CACHE is None:
        _NC_CACHE = build_nc()
    nc = _NC_CACHE
    in_maps = prep_inputs(inputs)

    import os
    trace = bool(os.environ.get("KERNEL_TRACE"))
    if trace:
        _ensure_ntff_hook()
    res = run_bass_kernel_spmd(nc, in_maps, core_ids=list(range(N_CORES)), trace=trace)
    LAST_RESULTS = res

    outs = np.stack([r["out_x"] for r in res.results])       # [8, 88, 250]
    x = outs.reshape(PAD_RANGES * K)                         # range = 88c + row
    return np.ascontiguousarray(x[:NUM_RANGES * K].reshape(NUM_RANGES * K, 1)).astype(np.float32)


# revision 26
# speedup vs baseline: 5.6444x; 1.1158x over previous
"""Trainium2 Bass kernel for MultiLayerRangeAttention (700 ranges x 250 keys, 2 layers).

Strategy (v2)
-------------
Data-parallel over ranges: pad 700 -> 704, 88 ranges per core, processed as 22
quads of 4 ranges. All matmuls in bf16 (fp32 matmul is emulated 2-pass on TRN2).
Transposed layout per quad: hT4 [32, 250] holds 4 ranges (8 rows each:
x, 6 embed rows, ones). Per layer l and quad:

  q/k:   W_bd4 [32,128] blockdiag -> PSUM [128, 500] (q | k in free),
         4 ranges stacked 32-partitions apart; copy+cast to bf16.
  vcol:  hT4-chunk lhsT x Wv_bd4 -> [v_r | ones] column pairs in PSUM scratch.
  scores(T): per range r at tile_position (32r, 0) -> 4-way concurrent MMs
         into a 4-bank PSUM mega tile [128, 2048] (range r in bank r).
  exp:   one ACT instr over a strided AP [125, 4, 500] -> bf16 SBUF.
  num/den: accumulating [v|ones] MMs at tile_position (0, 32r) ->
         scratch rows {32r, 32r+1}; DMA-gathered into per-layer staging
         tiles (num rows, den rows) across all 88 ranges.

Phase-structured division: after all 22 quads of a layer, ONE DVE reciprocal
[88, 250] + ONE multiply produce all x2 values; a DMA scatters them back into
the hT4 tiles (layer 0) or out to DRAM (layer 1). This amortizes the expensive
DVE reciprocal (~6.5 cyc/elem) across all ranges.
"""

import numpy as np
import ml_dtypes

import concourse.bass as bass
import concourse.bacc as bacc
import concourse.tile as tile
from concourse.tile import add_dep_helper
from concourse import mybir
from concourse.bass_utils import run_bass_kernel_spmd

NUM_RANGES = 700
K = 250
HID = 32
NUM_LAYERS = 2
SCALE = 1.0 / (HID ** 0.5)

N_CORES = 8
RPC = 88                      # ranges per core
QUADS = RPC // 4              # 22
PAD_RANGES = N_CORES * RPC    # 704

F32 = mybir.dt.float32
BF16 = mybir.dt.bfloat16
HALF = K // 2                 # 125

# wconst layout (bf16): per layer l, cols [384*l, 384*l + 384):
#   [0:128)   Wq_bd4 (SCALE+bias folded)   [128:256) Wk_bd4 (bias folded)
#   [256:384) Wv_bd4_wide: group r at cols [32r, 32r+32) = [v_r | ones_r | 0...]
WCOLS = 768


def build_nc() -> bass.Bass:
    nc = bacc.Bacc()
    h_all = nc.dram_tensor("h_all", [QUADS, 32, K], BF16, kind="ExternalInput")[:]
    wconst = nc.dram_tensor("wconst", [32, WCOLS], BF16, kind="ExternalInput")[:]
    out_x = nc.dram_tensor("out_x", [RPC, K], F32, kind="ExternalOutput")[:]

    with tile.TileContext(nc) as tc:
        with (
            tc.tile_pool(name="wpool", bufs=1) as wpool,
            tc.tile_pool(name="hpool", bufs=QUADS) as hpool,
            tc.tile_pool(name="qksb", bufs=3) as qksb,
            tc.tile_pool(name="vcsb", bufs=3) as vcsb,
            tc.tile_pool(name="expool", bufs=3) as expool,
            tc.tile_pool(name="ndst", bufs=2) as ndst,
            tc.tile_pool(name="ndsbp", bufs=2 * QUADS) as ndsbp,
            tc.tile_pool(name="divp", bufs=2) as divp,
            tc.tile_pool(name="qkps", bufs=1, space="PSUM") as qkps,
            tc.tile_pool(name="scps", bufs=3, space="PSUM") as scps,
            tc.tile_pool(name="srps", bufs=1, space="PSUM") as srps,
        ):
            wsb = wpool.tile([32, WCOLS], BF16)
            nc.sync.dma_start(out=wsb, in_=wconst)

            hts = []
            for q in range(QUADS):
                ht = hpool.tile([32, K], BF16)
                nc.sync.dma_start(out=ht, in_=h_all[q])
                hts.append(ht)

            for l in range(NUM_LAYERS):
                wb = 384 * l
                # num at cols [0:250), den at cols [250:500), one row per range
                nd_all = ndst.tile([96, 2 * K], F32, tag="nd_all")
                for q in range(QUADS):
                    ht = hts[q]
                    # --- q/k projections ---
                    qk_ps = qkps.tile([128, 2 * K], F32)
                    nc.tensor.matmul(qk_ps[:, 0:K], lhsT=wsb[:, wb:wb + 128],
                                     rhs=ht, start=True, stop=True)
                    nc.tensor.matmul(qk_ps[:, K:2 * K], lhsT=wsb[:, wb + 128:wb + 256],
                                     rhs=ht, start=True, stop=True)
                    qk_sb = qksb.tile([128, 2 * K], BF16)
                    nc.vector.tensor_copy(qk_sb, qk_ps)

                    # --- scratch bank: vcol cols [250:506), nd rows at [0:128) ---
                    scratch = srps.tile([128, 512], F32)
                    nc.tensor.matmul(scratch[0:HALF, K:K + 128], lhsT=ht[:, 0:HALF],
                                     rhs=wsb[:, wb + 256:wb + 384], start=True, stop=True)
                    nc.tensor.matmul(scratch[0:HALF, K + 128:K + 256], lhsT=ht[:, HALF:K],
                                     rhs=wsb[:, wb + 256:wb + 384], start=True, stop=True)
                    vc_sb = vcsb.tile([HALF, 256], BF16)
                    nc.vector.tensor_copy(vc_sb, scratch[0:HALF, K:K + 256])

                    # --- scoresT: 4-way concurrent row groups, two 2-range PSUM tiles ---
                    exs = []
                    for h in range(2):
                        sc_ps = scps.tile([128, 1024], F32, tag="sc_ps")
                        sc_mms = []
                        for c in range(2):
                            for rr in range(2):
                                r = 2 * h + rr
                                b = 32 * r
                                mm = nc.tensor.matmul(
                                    sc_ps[0:HALF, 512 * rr + 250 * c:512 * rr + 250 * c + 250],
                                    lhsT=qk_sb[b:b + 32, K + HALF * c:K + HALF * c + HALF],
                                    rhs=qk_sb[b:b + 32, 0:K],
                                    start=True, stop=True, tile_position=(b, 0))
                                sc_mms.append(mm)
                        ex = expool.tile([HALF, 2, 2 * K], BF16, tag="ex")
                        sc_view = sc_ps[0:HALF, :].rearrange("p (b x) -> p b x", x=512)[:, :, 0:2 * K]
                        act = nc.scalar.activation(ex, sc_view, mybir.ActivationFunctionType.Exp)
                        for mm in sc_mms:
                            add_dep_helper(act.ins, mm.ins, sync=True, reason="exp after scores")
                        exs.append(ex)

                    # --- num/den: [v | ones | 0...] accumulating MMs, 4-way col groups ---
                    nd_mms = []
                    for r in range(4):
                        for c in range(2):
                            b = 32 * r
                            mm = nc.tensor.matmul(
                                scratch[b:b + 32, 0:K],
                                lhsT=vc_sb[:, 128 * c + b:128 * c + b + 32],
                                rhs=exs[r // 2][:, r % 2, 250 * c:250 * c + 250],
                                start=(c == 0), stop=(c == 1), tile_position=(0, b))
                            nd_mms.append(mm)
                    # --- evacuate num/den rows: PSUM -> SBUF copy, then ONE DMA gather ---
                    ndsb = ndsbp.tile([128, K], F32)
                    cp = nc.vector.tensor_copy(ndsb, scratch[:, 0:K])
                    for mm in nd_mms:
                        add_dep_helper(cp.ins, mm.ins, sync=True, reason="ndsb copy after nd")
                    for r in range(4):
                        nc.sync.dma_start(out=nd_all[4 * q + r:4 * q + r + 1, :],
                                          in_=ndsb[32 * r:32 * r + 2, :])

                # --- phase: all divisions for this layer at once ---
                rden = divp.tile([RPC, K], F32, tag="rden")
                nc.vector.reciprocal(rden, nd_all[0:RPC, K:2 * K])
                if l == 0:
                    x2b = divp.tile([RPC, K], BF16, tag="x2b")
                    nc.vector.tensor_mul(x2b, nd_all[0:RPC, 0:K], rden)
                    for q in range(QUADS):
                        nc.sync.dma_start(out=hts[q][0:32:8, :],
                                          in_=x2b[4 * q:4 * q + 4, :])
                else:
                    x2f = divp.tile([RPC, K], F32, tag="x2f")
                    nc.vector.tensor_mul(x2f, nd_all[0:RPC, 0:K], rden)
                    nc.sync.dma_start(out=out_x, in_=x2f)
    nc.compile()
    return nc


def build_wconst(Wq, bq, Wk, bk, Wv, bv) -> np.ndarray:
    w = np.zeros((32, WCOLS), np.float32)
    for l in range(NUM_LAYERS):
        base = 384 * l
        wq = Wq[l] * SCALE
        bq_l = bq[l] * SCALE
        for (mat, bias, off) in ((wq, bq_l, 0), (Wk[l], bk[l], 128)):
            blk = w[:, base + off:base + off + 128]
            for r in range(4):
                cols = slice(32 * r, 32 * r + 32)
                blk[8 * r + 0, cols] = mat[0]
                blk[8 * r + 1:8 * r + 7, cols] = mat[1:7]
                blk[8 * r + 7, cols] = bias
        vb = w[:, base + 256:base + 384]
        for r in range(4):
            vb[8 * r + 0, 32 * r] = Wv[l][0, 0]
            vb[8 * r + 1:8 * r + 7, 32 * r] = Wv[l][1:7, 0]
            vb[8 * r + 7, 32 * r] = bv[l][0]
            vb[8 * r + 7, 32 * r + 1] = 1.0
    return w.astype(ml_dtypes.bfloat16)


def prep_inputs(inputs) -> list[dict]:
    pv = np.ascontiguousarray(np.asarray(inputs["power_vals"], np.float32)).reshape(-1)
    ele = np.asarray(inputs["ele_indices"]).astype(np.int64)
    azi = np.asarray(inputs["azi_indices"]).astype(np.int64)
    e = np.asarray(inputs["ele_emb"], np.float32)[ele]   # [N, 3]
    a = np.asarray(inputs["azi_emb"], np.float32)[azi]   # [N, 3]
    n = NUM_RANGES * K
    feats = np.empty((PAD_RANGES * K, 8), np.float32)
    feats[n:] = 0.0
    feats[:n, 0] = pv
    feats[:n, 1:4] = e
    feats[:n, 4:7] = a
    feats[:, 7] = 1.0
    # [704, 250, 8] -> [704, 8, 250] -> quads [176, 4, 8, 250] -> [176, 32, 250]
    fT = feats.reshape(PAD_RANGES, K, 8).transpose(0, 2, 1)
    hT = np.ascontiguousarray(
        fT.reshape(PAD_RANGES // 4, 4 * 8, K)).astype(ml_dtypes.bfloat16)

    wconst = build_wconst(np.asarray(inputs["Wq"], np.float32),
                          np.asarray(inputs["bq"], np.float32),
                          np.asarray(inputs["Wk"], np.float32),
                          np.asarray(inputs["bk"], np.float32),
                          np.asarray(inputs["Wv"], np.float32),
                          np.asarray(inputs["bv"], np.float32))
    return [
        {"h_all": np.ascontiguousarray(hT[QUADS * c:QUADS * (c + 1)]),
         "wconst": wconst}
        for c in range(N_CORES)
    ]


_NC_CACHE = None
LAST_RESULTS = None


def _ensure_ntff_hook():
    """The agent image's ``antenv`` lacks ``axon_hooks``, so trn_boot's NTFF
    hook registration degrades silently. Recreate the module + hook here so
    ``run_bass_kernel_spmd(trace=True)`` can capture HW exec times."""
    import sys
    import types
    try:
        from antenv.axon_hooks import get_axon_ntff_profile_hook  # noqa: F401
        return
    except ImportError:
        pass
    try:
        from trn_agent_boot.trn_boot import _ntff_profile_via_ctypes
        hook = _ntff_profile_via_ctypes("/opt/axon/libaxon_pjrt.so")
        mod = types.ModuleType("antenv.axon_hooks")
        mod._hook = hook
        mod.get_axon_ntff_profile_hook = lambda: mod._hook
        mod.set_axon_ntff_profile_hook = lambda h: setattr(mod, "_hook", h)
        import antenv
        antenv.axon_hooks = mod
        sys.modules["antenv.axon_hooks"] = mod
    except Exception as ex:
        print(f"NTFF hook setup failed ({ex}); running without trace", flush=True)


def kernel(**inputs) -> np.ndarray:
    global _NC_CACHE, LAST_RESULTS
    if _NC_CACHE is None:
        _NC_CACHE = build_nc()
    nc = _NC_CACHE
    in_maps = prep_inputs(inputs)

    import os
    trace = bool(os.environ.get("KERNEL_TRACE"))
    if trace:
        _ensure_ntff_hook()
    res = run_bass_kernel_spmd(nc, in_maps, core_ids=list(range(N_CORES)), trace=trace)
    LAST_RESULTS = res

    outs = np.stack([r["out_x"] for r in res.results])       # [8, 88, 250]
    x = outs.reshape(PAD_RANGES * K)                         # range = 88c + row
    return np.ascontiguousarray(x[:NUM_RANGES * K].reshape(NUM_RANGES * K, 1)).astype(np.float32)
